# revision 9
# baseline (speedup 1.0000x reference)
"""ApproxCompressor Trainium2 kernel (8 NeuronCores, data parallel over batch).

Algorithm: the reference's FFT convolution with the truncated exponential
impulse response h[n] = (1-a) a^n is a one-pole IIR y[t] = a y[t-1] + (1-a) e[t]
minus a tail term a^16384 y[t-16384] that underflows to zero in float32 for
any alpha = sigmoid(randn).  On-device we therefore run an exact recursive
scan instead of an FFT:

  - each core gets 4 examples; each example's L=131072 samples are laid out
    as 32 partition-chunks x 4096, so all 128 partitions scan in parallel
    (DVE tensor_tensor_scan along the free dim)
  - the cross-chunk scan carries are fixed up post-hoc: carry[p] is a linear
    function of the per-chunk final values S (carry = M @ S, M precomputed on
    host in f64), applied as a rank-1 update carry[p] * a^(i+1) via two tiny
    TensorE matmuls; a^(i+1) underflows past ~600 samples so only the first
    nb*512 columns of each chunk need fixing
  - the quadratic-knee gain is refactored into per-partition-scalar ops:
      d    = ln(e^{-thr} * (y + eps))            (ACT, scale/bias fold)
      u    = clamp(d, -W, W)                     (DVE tensor_scalar)
      sqv  = (s*u + s*W)^2,  s = sqrt(-c/(4W))   (ACT square)
      dm   = (d - W) * (-c)                      (DVE tensor_scalar)
      comb = max(dm, 0) + sqv                    (DVE scalar_tensor_tensor)
      gain = exp(-comb)                          (ACT)
    which equals exp(c*q(d)) of the reference knee exactly
  - out_c = gain * x_c on DVE / GPSIMD
"""

import numpy as np

N, C, L = 32, 2, 131072
NCORES = 8
NE = N // NCORES          # examples per core
CH = 32                   # partition-chunks per example
P = NE * CH               # 128 partitions
F = L // CH               # 4096 samples per partition
FH = 2048                 # head (square/scan) chunk width
FT = 1024                 # tail (knee/gain) chunk width
BANK = 512                # psum bank width for the carry fix
EPS = 1e-5

_CACHE = {}


def _build(nb, sim_init=False):
    import concourse.bass as bass
    import concourse.tile as tile
    from concourse import bacc, mybir

    f32 = mybir.dt.float32
    AF = mybir.ActivationFunctionType
    OP = mybir.AluOpType

    nc = bacc.Bacc("TRN2", target_bir_lowering=False, debug=False, num_devices=NCORES)

    x_h = nc.declare_dram_parameter("x", [NE, C, L], f32, isOutput=False)
    scal_h = nc.declare_dram_parameter("scal", [P, 16], f32, isOutput=False)
    mmt_h = nc.declare_dram_parameter("mmt", [P, P], f32, isOutput=False)
    ind_h = nc.declare_dram_parameter("ind", [P, NE], f32, isOutput=False)
    dec_h = nc.declare_dram_parameter("decay", [NE, nb * BANK], f32, isOutput=False)
    out_h = nc.declare_dram_parameter("out", [NE, C, L], f32, isOutput=True)

    # per-channel DRAM views: [c, e, k, i]
    xr = x_h[:].rearrange("e c (k i) -> c e k i", k=CH)
    outr = out_h[:].rearrange("e c (k i) -> c e k i", k=CH)

    from contextlib import ExitStack

    with tile.TileContext(nc) as tc, ExitStack() as ctx:
        const = ctx.enter_context(tc.tile_pool(name="const", bufs=1))
        res = ctx.enter_context(tc.tile_pool(name="res", bufs=1))
        hwork = ctx.enter_context(tc.tile_pool(name="hwork", bufs=2))
        twork = ctx.enter_context(tc.tile_pool(name="twork", bufs=2))
        psum = ctx.enter_context(tc.tile_pool(name="psum", bufs=2, space="PSUM"))

        scal_t = const.tile([P, 16], f32)
        nc.sync.dma_start(scal_t[:], scal_h[:])
        mmt_t = const.tile([P, P], f32)
        nc.sync.dma_start(mmt_t[:], mmt_h[:])
        ind_t = const.tile([P, NE], f32)
        nc.sync.dma_start(ind_t[:], ind_h[:])
        dec_t = const.tile([NE, nb * BANK], f32)
        nc.sync.dma_start(dec_t[:], dec_h[:])

        # broadcast alpha along the free dim for the scan's data0
        alpha_t = const.tile([P, FH], f32)
        nc.vector.memset(alpha_t[:], 1.0)
        nc.vector.tensor_scalar_mul(alpha_t[:], alpha_t[:], scal_t[:, 1:2])

        # resident input tiles: one whole tile per (channel, head-chunk) so
        # every DMA write covers a full tile (keeps dep tracking exact)
        NH = F // FH
        x_t = [[res.tile([P, FH], f32, tag=f"x{c}h{h}", name=f"x{c}h{h}")
                for h in range(NH)] for c in range(C)]
        y_t = res.tile([P, F], f32)

        s1 = scal_t[:, 0:1]
        # ---- head: energy + local scans ----
        for f in range(NH):
            sl = slice(f * FH, (f + 1) * FH)
            for c in range(C):
                # SBUF side stays 2-D (whole tile); DRAM side is 3-D with the
                # same element iteration order (p = e*CH + k)
                nc.sync.dma_start(x_t[c][f][:], xr[c][:, :, sl])
            sq0 = hwork.tile([P, FH], f32, tag="sq0")
            sq1 = hwork.tile([P, FH], f32, tag="sq1")
            nc.scalar.activation(sq0[:], x_t[0][f][:], AF.Square, bias=0.0, scale=s1)
            nc.scalar.activation(sq1[:], x_t[1][f][:], AF.Square, bias=0.0, scale=s1)
            e_t = hwork.tile([P, FH], f32, tag="e")
            nc.gpsimd.tensor_tensor(e_t[:], sq0[:], sq1[:], op=OP.add)
            init = 0.0 if f == 0 else y_t[:, f * FH - 1 : f * FH]
            nc.vector.tensor_tensor_scan(
                y_t[:, sl], alpha_t[:], e_t[:], init, op0=OP.mult, op1=OP.add
            )

        # ---- carry fix ----
        s_lhs = const.tile([P, NE], f32)
        nc.vector.tensor_scalar_mul(s_lhs[:], ind_t[:], y_t[:, F - 1 : F])
        p1 = psum.tile([NE, P], f32, tag="p1")
        nc.tensor.matmul(p1[:], s_lhs[:], mmt_t[:], start=True, stop=True)
        carryT = const.tile([NE, P], f32)
        nc.scalar.copy(carryT[:], p1[:])
        for b in range(nb):
            pc = psum.tile([P, BANK], f32, tag="pc")
            nc.tensor.matmul(
                pc[:], carryT[:], dec_t[:, b * BANK : (b + 1) * BANK],
                start=True, stop=True,
            )
            ysl = y_t[:, b * BANK : (b + 1) * BANK]
            nc.vector.tensor_add(ysl, ysl, pc[:])

        # ---- tail: knee gain + output ----
        lnscale, lnbias = scal_t[:, 2:3], scal_t[:, 3:4]
        negW, W_c = scal_t[:, 4:5], scal_t[:, 5:6]
        negc, s_c, sW = scal_t[:, 6:7], scal_t[:, 7:8], scal_t[:, 8:9]
        for f in range(F // FT):
            sl = slice(f * FT, (f + 1) * FT)
            d_t = twork.tile([P, FT], f32, tag="d")
            nc.scalar.activation(d_t[:], y_t[:, sl], AF.Ln, bias=lnbias, scale=lnscale)
            u_t = twork.tile([P, FT], f32, tag="u")
            nc.vector.tensor_scalar(u_t[:], d_t[:], negW, W_c, op0=OP.max, op1=OP.min)
            sqv_t = twork.tile([P, FT], f32, tag="sqv")
            nc.scalar.activation(sqv_t[:], u_t[:], AF.Square, bias=sW, scale=s_c)
            dm_t = twork.tile([P, FT], f32, tag="dm")
            nc.vector.tensor_scalar(
                dm_t[:], d_t[:], W_c, negc, op0=OP.subtract, op1=OP.mult
            )
            comb_t = twork.tile([P, FT], f32, tag="comb")
            nc.vector.scalar_tensor_tensor(
                comb_t[:], dm_t[:], 0.0, sqv_t[:], op0=OP.max, op1=OP.add
            )
            g_t = twork.tile([P, FT], f32, tag="g")
            nc.scalar.activation(g_t[:], comb_t[:], AF.Exp, bias=0.0, scale=-1.0)
            o0_t = twork.tile([P, FT], f32, tag="o0")
            o1_t = twork.tile([P, FT], f32, tag="o1")
            h, hsl = f // (FH // FT), slice((f % (FH // FT)) * FT,
                                            (f % (FH // FT)) * FT + FT)
            nc.vector.tensor_mul(o0_t[:], g_t[:], x_t[0][h][:, hsl])
            nc.gpsimd.tensor_tensor(o1_t[:], g_t[:], x_t[1][h][:, hsl], op=OP.mult)
            nc.sync.dma_start(outr[0][:, :, sl], o0_t[:])
            nc.sync.dma_start(outr[1][:, :, sl], o1_t[:])

    nc.compile()
    return nc


def _host_consts(lt, lr, lk, za, nb):
    """Per-core constant tensors from the [NE] parameter vectors (f64 math)."""
    alpha = 1.0 / (1.0 + np.exp(-za))
    thr = lt - 6.0
    r = 1.0 + np.exp(lr)
    c = 1.0 / r - 1.0
    W = np.exp(lk) / 2.0

    cols = np.zeros((NE, 16))
    cols[:, 0] = np.sqrt((1.0 - alpha) / 2.0)     # s1: energy scale
    cols[:, 1] = alpha
    cols[:, 2] = np.exp(-thr)                     # lnscale
    cols[:, 3] = EPS * np.exp(-thr)               # lnbias
    cols[:, 4] = -W
    cols[:, 5] = W
    cols[:, 6] = -c
    cols[:, 7] = np.sqrt(-c / (4.0 * W))          # s
    cols[:, 8] = np.sqrt(-c / (4.0 * W)) * W      # s*W
    scal = np.repeat(cols, CH, axis=0).astype(np.float32)   # [P, 16]

    # carry matrix: carry[p] = sum_{q<p, same example} A^(p-1-q) S[q]
    A = alpha**F
    mmt = np.zeros((P, P))
    for e in range(NE):
        for pp in range(CH):
            for q in range(pp):
                mmt[CH * e + q, CH * e + pp] = A[e] ** (pp - 1 - q)  # [q, p] = M[p, q]
    mmt = mmt.astype(np.float32)

    ind = np.repeat(np.eye(NE), CH, axis=0).astype(np.float32)  # [P, NE]
    dec = (alpha[:, None] ** np.arange(1, nb * BANK + 1)[None, :]).astype(np.float32)
    return {"scal": scal, "mmt": mmt, "ind": ind, "decay": dec}


def _pick_nb(za):
    alpha_max = float(1.0 / (1.0 + np.exp(-np.max(za))))
    alpha_max = min(max(alpha_max, 1e-6), 1.0 - 1e-9)
    need = np.log(1e-12) / np.log(alpha_max)
    return int(min(max(np.ceil(need / BANK), 1), F // BANK))


def _prep(inputs):
    x = np.ascontiguousarray(np.asarray(inputs["input_signals"], np.float32))
    lt = np.asarray(inputs["log_threshold"], np.float64).reshape(N)
    lr = np.asarray(inputs["log_ratio"], np.float64).reshape(N)
    lk = np.asarray(inputs["log_knee"], np.float64).reshape(N)
    za = np.asarray(inputs["z_alpha_pre"], np.float64).reshape(N)
    nb = _pick_nb(za)
    in_maps = []
    for i in range(NCORES):
        s = slice(i * NE, (i + 1) * NE)
        m = {"x": x[s]}
        m.update(_host_consts(lt[s], lr[s], lk[s], za[s], nb))
        in_maps.append(m)
    return nb, in_maps


def _get_nc(nb):
    if nb not in _CACHE:
        _CACHE[nb] = _build(nb)
    return _CACHE[nb]


def _run(inputs, trace=False):
    from concourse.bass_utils import run_bass_kernel_spmd

    nb, in_maps = _prep(inputs)
    nc = _get_nc(nb)
    res = run_bass_kernel_spmd(nc, in_maps, core_ids=list(range(NCORES)), trace=trace)
    out = np.concatenate([res.results[i]["out"] for i in range(NCORES)], axis=0)
    return out, res


def kernel(**inputs):
    out, _ = _run(inputs, trace=False)
    return out


# revision 10
# speedup vs baseline: 1.1232x; 1.1232x over previous
"""ApproxCompressor Trainium2 kernel (8 NeuronCores, data parallel over batch).

Algorithm: the reference's FFT convolution with the truncated exponential
impulse response h[n] = (1-a) a^n is a one-pole IIR y[t] = a y[t-1] + (1-a) e[t]
minus a tail term a^16384 y[t-16384] that underflows to zero in float32 for
any alpha = sigmoid(randn).  On-device we therefore run an exact recursive
scan instead of an FFT:

  - each core gets 4 examples; each example's L=131072 samples are laid out
    as 32 partition-chunks x 4096, so all 128 partitions scan in parallel
    (DVE tensor_tensor_scan along the free dim)
  - the cross-chunk scan carries are fixed up post-hoc: carry[p] is a linear
    function of the per-chunk final values S (carry = M @ S, M precomputed on
    host in f64), applied as a rank-1 update carry[p] * a^(i+1) via two tiny
    TensorE matmuls; a^(i+1) underflows past ~600 samples so only the first
    nb*512 columns of each chunk need fixing
  - the quadratic-knee gain is refactored into per-partition-scalar ops:
      d    = ln(e^{-thr} * (y + eps))            (ACT, scale/bias fold)
      u    = clamp(d, -W, W)                     (DVE tensor_scalar)
      sqv  = (s*u + s*W)^2,  s = sqrt(-c/(4W))   (ACT square)
      dm   = (d - W) * (-c)                      (DVE tensor_scalar)
      comb = max(dm, 0) + sqv                    (DVE scalar_tensor_tensor)
      gain = exp(-comb)                          (ACT)
    which equals exp(c*q(d)) of the reference knee exactly
  - out_c = gain * x_c on DVE / GPSIMD
"""

import numpy as np

N, C, L = 32, 2, 131072
NCORES = 8
NE = N // NCORES          # examples per core
CH = 32                   # partition-chunks per example
P = NE * CH               # 128 partitions
F = L // CH               # 4096 samples per partition
FH = 2048                 # head (square/scan) chunk width
FT = 1024                 # tail (knee/gain) chunk width
BANK = 512                # psum bank width for the carry fix
EPS = 1e-5

_CACHE = {}


def _build(nb, sim_init=False):
    import concourse.bass as bass
    import concourse.tile as tile
    from concourse import bacc, mybir

    f32 = mybir.dt.float32
    AF = mybir.ActivationFunctionType
    OP = mybir.AluOpType

    nc = bacc.Bacc("TRN2", target_bir_lowering=False, debug=False, num_devices=NCORES)

    x_h = nc.declare_dram_parameter("x", [NE, C, L], f32, isOutput=False)
    scal_h = nc.declare_dram_parameter("scal", [P, 16], f32, isOutput=False)
    mmt_h = nc.declare_dram_parameter("mmt", [P, P], f32, isOutput=False)
    ind_h = nc.declare_dram_parameter("ind", [P, NE], f32, isOutput=False)
    dec_h = nc.declare_dram_parameter("decay", [NE, nb * BANK], f32, isOutput=False)
    out_h = nc.declare_dram_parameter("out", [NE, C, L], f32, isOutput=True)

    # per-channel DRAM views: [c, e, k, i]
    xr = x_h[:].rearrange("e c (k i) -> c e k i", k=CH)
    outr = out_h[:].rearrange("e c (k i) -> c e k i", k=CH)

    from contextlib import ExitStack

    with tile.TileContext(nc) as tc, ExitStack() as ctx:
        const = ctx.enter_context(tc.tile_pool(name="const", bufs=1))
        res = ctx.enter_context(tc.tile_pool(name="res", bufs=1))
        hwork = ctx.enter_context(tc.tile_pool(name="hwork", bufs=2))
        twork = ctx.enter_context(tc.tile_pool(name="twork", bufs=2))
        psum = ctx.enter_context(tc.tile_pool(name="psum", bufs=2, space="PSUM"))

        scal_t = const.tile([P, 16], f32)
        nc.sync.dma_start(scal_t[:], scal_h[:])
        mmt_t = const.tile([P, P], f32)
        nc.sync.dma_start(mmt_t[:], mmt_h[:])
        ind_t = const.tile([P, NE], f32)
        nc.sync.dma_start(ind_t[:], ind_h[:])
        dec_t = const.tile([NE, nb * BANK], f32)
        nc.sync.dma_start(dec_t[:], dec_h[:])


        # resident input tiles: one whole tile per (channel, head-chunk) so
        # every DMA write covers a full tile (keeps dep tracking exact)
        NH = F // FH
        x_t = [[res.tile([P, FH], f32, tag=f"x{c}h{h}", name=f"x{c}h{h}")
                for h in range(NH)] for c in range(C)]
        y_t = res.tile([P, F], f32)

        s1 = scal_t[:, 0:1]
        # ---- head: energy + local scans ----
        for f in range(NH):
            sl = slice(f * FH, (f + 1) * FH)
            for c in range(C):
                # SBUF side stays 2-D (whole tile); DRAM side is 3-D with the
                # same element iteration order (p = e*CH + k)
                nc.gpsimd.dma_start(x_t[c][f][:], xr[c][:, :, sl])
            sq0 = hwork.tile([P, FH], f32, tag="sq0")
            sq1 = hwork.tile([P, FH], f32, tag="sq1")
            nc.scalar.activation(sq0[:], x_t[0][f][:], AF.Square, bias=0.0, scale=s1)
            nc.scalar.activation(sq1[:], x_t[1][f][:], AF.Square, bias=0.0, scale=s1)
            e_t = hwork.tile([P, FH], f32, tag="e")
            nc.vector.tensor_tensor(e_t[:], sq0[:], sq1[:], op=OP.add)
            init = 0.0 if f == 0 else y_t[:, f * FH - 1 : f * FH]
            nc.vector.tensor_tensor_scan(
                y_t[:, sl], scal_t[:, 1:2].broadcast_to([P, FH]), e_t[:], init,
                op0=OP.mult, op1=OP.add,
            )

        # ---- carry fix ----
        s_lhs = const.tile([P, NE], f32)
        nc.vector.tensor_scalar_mul(s_lhs[:], ind_t[:], y_t[:, F - 1 : F])
        p1 = psum.tile([NE, P], f32, tag="p1")
        nc.tensor.matmul(p1[:], s_lhs[:], mmt_t[:], start=True, stop=True)
        carryT = const.tile([NE, P], f32)
        nc.scalar.copy(carryT[:], p1[:])
        for b in range(nb):
            pc = psum.tile([P, BANK], f32, tag="pc")
            nc.tensor.matmul(
                pc[:], carryT[:], dec_t[:, b * BANK : (b + 1) * BANK],
                start=True, stop=True,
            )
            ysl = y_t[:, b * BANK : (b + 1) * BANK]
            nc.vector.tensor_add(ysl, ysl, pc[:])

        # ---- tail: knee gain + output ----
        lnscale, lnbias = scal_t[:, 2:3], scal_t[:, 3:4]
        negW, W_c = scal_t[:, 4:5], scal_t[:, 5:6]
        negc, s_c, sW = scal_t[:, 6:7], scal_t[:, 7:8], scal_t[:, 8:9]
        for f in [*range(1, F // FT), 0]:
            sl = slice(f * FT, (f + 1) * FT)
            d_t = twork.tile([P, FT], f32, tag="d")
            nc.scalar.activation(d_t[:], y_t[:, sl], AF.Ln, bias=lnbias, scale=lnscale)
            u_t = twork.tile([P, FT], f32, tag="u")
            nc.vector.tensor_scalar(u_t[:], d_t[:], negW, W_c, op0=OP.max, op1=OP.min)
            sqv_t = twork.tile([P, FT], f32, tag="sqv")
            nc.scalar.activation(sqv_t[:], u_t[:], AF.Square, bias=sW, scale=s_c)
            dm_t = twork.tile([P, FT], f32, tag="dm")
            nc.vector.tensor_scalar(
                dm_t[:], d_t[:], W_c, negc, op0=OP.subtract, op1=OP.mult
            )
            comb_t = twork.tile([P, FT], f32, tag="comb")
            nc.vector.scalar_tensor_tensor(
                comb_t[:], dm_t[:], 0.0, sqv_t[:], op0=OP.max, op1=OP.add
            )
            g_t = twork.tile([P, FT], f32, tag="g")
            nc.scalar.activation(g_t[:], comb_t[:], AF.Exp, bias=0.0, scale=-1.0)
            o0_t = twork.tile([P, FT], f32, tag="o0")
            o1_t = twork.tile([P, FT], f32, tag="o1")
            h, hsl = f // (FH // FT), slice((f % (FH // FT)) * FT,
                                            (f % (FH // FT)) * FT + FT)
            nc.vector.tensor_mul(o0_t[:], g_t[:], x_t[0][h][:, hsl])
            nc.vector.tensor_tensor(o1_t[:], g_t[:], x_t[1][h][:, hsl], op=OP.mult)
            nc.gpsimd.dma_start(outr[0][:, :, sl], o0_t[:])
            nc.gpsimd.dma_start(outr[1][:, :, sl], o1_t[:])

    nc.compile()
    return nc


def _host_consts(lt, lr, lk, za, nb):
    """Per-core constant tensors from the [NE] parameter vectors (f64 math)."""
    alpha = 1.0 / (1.0 + np.exp(-za))
    thr = lt - 6.0
    r = 1.0 + np.exp(lr)
    c = 1.0 / r - 1.0
    W = np.exp(lk) / 2.0

    cols = np.zeros((NE, 16))
    cols[:, 0] = np.sqrt((1.0 - alpha) / 2.0)     # s1: energy scale
    cols[:, 1] = alpha
    cols[:, 2] = np.exp(-thr)                     # lnscale
    cols[:, 3] = EPS * np.exp(-thr)               # lnbias
    cols[:, 4] = -W
    cols[:, 5] = W
    cols[:, 6] = -c
    cols[:, 7] = np.sqrt(-c / (4.0 * W))          # s
    cols[:, 8] = np.sqrt(-c / (4.0 * W)) * W      # s*W
    scal = np.repeat(cols, CH, axis=0).astype(np.float32)   # [P, 16]

    # carry matrix: carry[p] = sum_{q<p, same example} A^(p-1-q) S[q]
    A = alpha**F
    mmt = np.zeros((P, P))
    for e in range(NE):
        for pp in range(CH):
            for q in range(pp):
                mmt[CH * e + q, CH * e + pp] = A[e] ** (pp - 1 - q)  # [q, p] = M[p, q]
    mmt = mmt.astype(np.float32)

    ind = np.repeat(np.eye(NE), CH, axis=0).astype(np.float32)  # [P, NE]
    dec = (alpha[:, None] ** np.arange(1, nb * BANK + 1)[None, :]).astype(np.float32)
    return {"scal": scal, "mmt": mmt, "ind": ind, "decay": dec}


def _pick_nb(za):
    alpha_max = float(1.0 / (1.0 + np.exp(-np.max(za))))
    alpha_max = min(max(alpha_max, 1e-6), 1.0 - 1e-9)
    need = np.log(1e-12) / np.log(alpha_max)
    return int(min(max(np.ceil(need / BANK), 1), F // BANK))


def _prep(inputs):
    x = np.ascontiguousarray(np.asarray(inputs["input_signals"], np.float32))
    lt = np.asarray(inputs["log_threshold"], np.float64).reshape(N)
    lr = np.asarray(inputs["log_ratio"], np.float64).reshape(N)
    lk = np.asarray(inputs["log_knee"], np.float64).reshape(N)
    za = np.asarray(inputs["z_alpha_pre"], np.float64).reshape(N)
    nb = _pick_nb(za)
    in_maps = []
    for i in range(NCORES):
        s = slice(i * NE, (i + 1) * NE)
        m = {"x": x[s]}
        m.update(_host_consts(lt[s], lr[s], lk[s], za[s], nb))
        in_maps.append(m)
    return nb, in_maps


def _get_nc(nb):
    if nb not in _CACHE:
        _CACHE[nb] = _build(nb)
    return _CACHE[nb]


def _run(inputs, trace=False):
    from concourse.bass_utils import run_bass_kernel_spmd

    nb, in_maps = _prep(inputs)
    nc = _get_nc(nb)
    res = run_bass_kernel_spmd(nc, in_maps, core_ids=list(range(NCORES)), trace=trace)
    out = np.concatenate([res.results[i]["out"] for i in range(NCORES)], axis=0)
    return out, res


def kernel(**inputs):
    out, _ = _run(inputs, trace=False)
    return out


# revision 16
# speedup vs baseline: 1.7650x; 1.5715x over previous
"""ApproxCompressor Trainium2 kernel (8 NeuronCores, data parallel over batch).

Algorithm: the reference's FFT convolution with the truncated exponential
impulse response h[n] = (1-a) a^n is a one-pole IIR y[t] = a y[t-1] + (1-a) e[t]
minus a tail term a^16384 y[t-16384] that underflows to zero in float32 for
any alpha = sigmoid(randn).  On-device we therefore run an exact recursive
scan instead of an FFT.

Per core: 4 examples, processed as a 4-deep pipeline.  Each example's
L=131072 samples are laid out as [128 partitions x 1024], so every DMA is a
fully contiguous 512KB HBM transfer (strided patterns measured ~80GB/s vs
~300GB/s contiguous) and all 128 partitions scan in parallel (DVE
tensor_tensor_scan along the free dim, one independent recurrence per
partition-chunk).

Cross-chunk scan carries are fixed post-hoc: carry[p] (the true initial
state of chunk p) is linear in the per-chunk final values S, carry = M @ S
with M precomputed on host in f64.  Two tiny TensorE matmuls compute
carryT = S^T @ M^T ([1,128]) then the rank-1 update carry[p] * a^(i+1)
([128, nb*512], a^(i+1) underflows past ~600 samples so nb banks suffice),
accumulated onto y by one DVE add.

The quadratic-knee gain is refactored into per-partition-scalar ops:
    d    = ln(e^{-thr} * (y + eps))            (ACT, scale/bias fold)
    u    = clamp(d, -W, W)                     (DVE tensor_scalar)
    sqv  = (s*u + s*W)^2,  s = sqrt(-c/(4W))   (ACT square)
    dm   = (d - W) * (-c)                      (DVE tensor_scalar)
    comb = max(dm, 0) + sqv                    (DVE scalar_tensor_tensor)
    gain = exp(-comb)                          (ACT)
which equals exp(c*q(d)) of the reference knee exactly.  out_c = gain * x_c
is computed in place into the input tiles, which are then DMA'd out
contiguously.  Ln/Exp/Square all live in the natural_log_exp_and_others
ACT table set; get_activation_tables is narrowed during compile so the
set chooser picks it (avoids per-chunk table reloads).
"""

import numpy as np

N, C, L = 32, 2, 131072
NCORES = 8
NE = N // NCORES          # examples per core
P = 128                   # partitions = chunks per example
F = L // P                # 1024 samples per partition
BANK = 512                # psum bank width for the carry fix
EPS = 1e-5

_CACHE = {}


def _build(nb):
    import concourse.bass as bass
    import concourse.tile as tile
    from concourse import bacc, mybir

    f32 = mybir.dt.float32
    AF = mybir.ActivationFunctionType
    OP = mybir.AluOpType

    nc = bacc.Bacc("TRN2", target_bir_lowering=False, debug=False, num_devices=NCORES)

    x_h = nc.declare_dram_parameter("x", [NE, C, L], f32, isOutput=False)
    scal_h = nc.declare_dram_parameter("scal", [P, 16 * NE], f32, isOutput=False)
    mmt_h = nc.declare_dram_parameter("mmt", [NE * P, P], f32, isOutput=False)
    dec_h = nc.declare_dram_parameter("decay", [1, NE * nb * BANK], f32, isOutput=False)
    out_h = nc.declare_dram_parameter("out", [NE, C, L], f32, isOutput=True)

    from contextlib import ExitStack

    with tile.TileContext(nc) as tc, ExitStack() as ctx:
        const = ctx.enter_context(tc.tile_pool(name="const", bufs=1))
        work = ctx.enter_context(tc.tile_pool(name="work", bufs=3))
        xpool = ctx.enter_context(tc.tile_pool(name="xpool", bufs=3))
        psum = ctx.enter_context(tc.tile_pool(name="psum", bufs=2, space="PSUM"))

        scal_t = const.tile([P, 16 * NE], f32)
        nc.sync.dma_start(scal_t[:], scal_h[:])
        mmt_t = [const.tile([P, P], f32, name=f"mmt{e}") for e in range(NE)]
        for e in range(NE):
            nc.sync.dma_start(mmt_t[e][:], mmt_h[:][e * P : (e + 1) * P, :])
        dec_t = const.tile([1, NE * nb * BANK], f32, padded_shape=[P, NE * nb * BANK])
        nc.sync.dma_start(dec_t[:], dec_h[:])

        def sc(e, j):
            return scal_t[:, 16 * e + j : 16 * e + j + 1]

        for e in range(NE):
            x0 = xpool.tile([P, F], f32, tag="x0")
            x1 = xpool.tile([P, F], f32, tag="x1")
            nc.gpsimd.dma_start(x0[:], x_h[:][e, 0].rearrange("(p i) -> p i", p=P))
            nc.gpsimd.dma_start(x1[:], x_h[:][e, 1].rearrange("(p i) -> p i", p=P))

            # energy e[t] = ((1-a)/2) * (x0^2 + x1^2), scale folded into the squares
            sq0 = work.tile([P, F], f32, tag="sq0")
            sq1 = work.tile([P, F], f32, tag="sq1")
            nc.scalar.activation(sq0[:], x0[:], AF.Square, bias=0.0, scale=sc(e, 0))
            nc.scalar.activation(sq1[:], x1[:], AF.Square, bias=0.0, scale=sc(e, 0))
            e_t = work.tile([P, F], f32, tag="e")
            nc.vector.tensor_tensor(e_t[:], sq0[:], sq1[:], op=OP.add)

            # local scans, one independent recurrence per partition-chunk
            y_t = work.tile([P, F], f32, tag="y")
            nc.vector.tensor_tensor_scan(
                y_t[:], sc(e, 1).broadcast_to([P, F]), e_t[:], 0.0,
                op0=OP.mult, op1=OP.add,
            )

            # carry fix: carryT = S^T @ M^T, then y[:, :nb*512] += carry x decay
            p1 = psum.tile([1, P], f32, tag="p1")
            nc.tensor.matmul(p1[:], y_t[:, F - 1 : F], mmt_t[e][:],
                             start=True, stop=True)
            carryT = work.tile([1, P], f32, tag="carryT", padded_shape=[P, P])
            nc.scalar.copy(carryT[:], p1[:])
            for b in range(nb):
                pc = psum.tile([P, BANK], f32, tag="pc")
                off = (e * nb + b) * BANK
                nc.tensor.matmul(
                    pc[:], carryT[:], dec_t[0:1, off : off + BANK],
                    start=True, stop=True,
                )
                ysl = y_t[:, b * BANK : (b + 1) * BANK]
                nc.vector.tensor_add(ysl, ysl, pc[:])

            # knee gain
            d_t = work.tile([P, F], f32, tag="d")
            nc.scalar.activation(d_t[:], y_t[:], AF.Ln, bias=sc(e, 3), scale=sc(e, 2))
            u_t = work.tile([P, F], f32, tag="u")
            nc.vector.tensor_scalar(u_t[:], d_t[:], sc(e, 4), sc(e, 5),
                                    op0=OP.max, op1=OP.min)
            sqv_t = work.tile([P, F], f32, tag="sqv")
            nc.scalar.activation(sqv_t[:], u_t[:], AF.Square,
                                 bias=sc(e, 8), scale=sc(e, 7))
            dm_t = work.tile([P, F], f32, tag="dm")
            nc.vector.tensor_scalar(dm_t[:], d_t[:], sc(e, 5), sc(e, 6),
                                    op0=OP.subtract, op1=OP.mult)
            comb_t = work.tile([P, F], f32, tag="comb")
            nc.vector.scalar_tensor_tensor(comb_t[:], dm_t[:], 0.0, sqv_t[:],
                                           op0=OP.max, op1=OP.add)
            g_t = work.tile([P, F], f32, tag="g")
            nc.scalar.activation(g_t[:], comb_t[:], AF.Exp, bias=0.0, scale=-1.0)

            # gain application in place, then contiguous DMA out
            nc.vector.tensor_mul(x0[:], g_t[:], x0[:])
            nc.vector.tensor_mul(x1[:], g_t[:], x1[:])
            nc.gpsimd.dma_start(out_h[:][e, 0].rearrange("(p i) -> p i", p=P), x0[:])
            nc.gpsimd.dma_start(out_h[:][e, 1].rearrange("(p i) -> p i", p=P), x1[:])

    # narrow the ACT table sets so Ln/Exp/Square resolve to the one set that
    # holds all three -> a single table load instead of per-chunk reloads
    import concourse.bacc as bacc_mod

    orig = bacc_mod.get_activation_tables
    strip = {AF.Ln, AF.Exp, AF.Square}

    def patched(arch):
        full = orig(arch)
        return {
            name: (set(fns) if name == "natural_log_exp_and_others"
                   else set(fns) - strip)
            for name, fns in full.items()
        }

    bacc_mod.get_activation_tables = patched
    try:
        nc.compile()
    finally:
        bacc_mod.get_activation_tables = orig
    return nc


def _host_consts(lt, lr, lk, za, nb):
    """Per-core constant tensors from the [NE] parameter vectors (f64 math)."""
    alpha = 1.0 / (1.0 + np.exp(-za))
    thr = lt - 6.0
    r = 1.0 + np.exp(lr)
    c = 1.0 / r - 1.0
    W = np.exp(lk) / 2.0

    cols = np.zeros((NE, 16))
    cols[:, 0] = np.sqrt((1.0 - alpha) / 2.0)     # s1: energy scale
    cols[:, 1] = alpha
    cols[:, 2] = np.exp(-thr)                     # lnscale
    cols[:, 3] = EPS * np.exp(-thr)               # lnbias
    cols[:, 4] = -W
    cols[:, 5] = W
    cols[:, 6] = -c
    cols[:, 7] = np.sqrt(-c / (4.0 * W))          # s
    cols[:, 8] = np.sqrt(-c / (4.0 * W)) * W      # s*W
    scal = np.tile(cols.reshape(1, NE * 16), (P, 1)).astype(np.float32)

    # carry matrix, transposed for the matmul: mmt[e][q, p] = A^(p-1-q), q < p
    A = alpha**F
    mmt = np.zeros((NE, P, P))
    qs = np.arange(P)
    for e in range(NE):
        for p in range(1, P):
            mmt[e, :p, p] = A[e] ** (p - 1 - qs[:p])
    mmt = mmt.reshape(NE * P, P).astype(np.float32)

    dec = (alpha[:, None] ** np.arange(1, nb * BANK + 1)[None, :]).astype(np.float32)
    return {"scal": scal, "mmt": mmt, "decay": dec.reshape(1, NE * nb * BANK)}


def _pick_nb(za):
    alpha_max = float(1.0 / (1.0 + np.exp(-np.max(za))))
    alpha_max = min(max(alpha_max, 1e-6), 1.0 - 1e-9)
    need = np.log(1e-10) / np.log(alpha_max)
    return int(min(max(np.ceil(need / BANK), 1), F // BANK))


def _prep(inputs):
    x = np.ascontiguousarray(np.asarray(inputs["input_signals"], np.float32))
    lt = np.asarray(inputs["log_threshold"], np.float64).reshape(N)
    lr = np.asarray(inputs["log_ratio"], np.float64).reshape(N)
    lk = np.asarray(inputs["log_knee"], np.float64).reshape(N)
    za = np.asarray(inputs["z_alpha_pre"], np.float64).reshape(N)
    nb = _pick_nb(za)
    in_maps = []
    for i in range(NCORES):
        s = slice(i * NE, (i + 1) * NE)
        m = {"x": x[s]}
        m.update(_host_consts(lt[s], lr[s], lk[s], za[s], nb))
        in_maps.append(m)
    return nb, in_maps


def _get_nc(nb):
    if nb not in _CACHE:
        _CACHE[nb] = _build(nb)
    return _CACHE[nb]


def _run(inputs, trace=False):
    from concourse.bass_utils import run_bass_kernel_spmd

    nb, in_maps = _prep(inputs)
    nc = _get_nc(nb)
    res = run_bass_kernel_spmd(nc, in_maps, core_ids=list(range(NCORES)), trace=trace)
    out = np.concatenate([res.results[i]["out"] for i in range(NCORES)], axis=0)
    return out, res


def kernel(**inputs):
    out, _ = _run(inputs, trace=False)
    return out


# revision 17
# speedup vs baseline: 2.0643x; 1.1696x over previous
"""ApproxCompressor Trainium2 kernel (8 NeuronCores, data parallel over batch).

Algorithm: the reference's FFT convolution with the truncated exponential
impulse response h[n] = (1-a) a^n is a one-pole IIR y[t] = a y[t-1] + (1-a) e[t]
minus a tail term a^16384 y[t-16384] that underflows to zero in float32 for
any alpha = sigmoid(randn).  On-device we therefore run an exact recursive
scan instead of an FFT.

Per core: 4 examples, processed as a 4-deep pipeline.  Each example's
L=131072 samples are laid out as [128 partitions x 1024], so every DMA is a
fully contiguous 512KB HBM transfer (strided patterns measured ~80GB/s vs
~300GB/s contiguous) and all 128 partitions scan in parallel (DVE
tensor_tensor_scan along the free dim, one independent recurrence per
partition-chunk).

Cross-chunk scan carries are fixed post-hoc: carry[p] (the true initial
state of chunk p) is linear in the per-chunk final values S, carry = M @ S
with M precomputed on host in f64.  Two tiny TensorE matmuls compute
carryT = S^T @ M^T ([1,128]) then the rank-1 update carry[p] * a^(i+1)
([128, nb*512], a^(i+1) underflows past ~600 samples so nb banks suffice),
accumulated onto y by one DVE add.

The quadratic-knee gain is refactored into per-partition-scalar ops:
    d    = ln(e^{-thr} * (y + eps))            (ACT, scale/bias fold)
    u    = clamp(d, -W, W)                     (DVE tensor_scalar)
    sqv  = (s*u + s*W)^2,  s = sqrt(-c/(4W))   (ACT square)
    dm   = (d - W) * (-c)                      (DVE tensor_scalar)
    comb = max(dm, 0) + sqv                    (DVE scalar_tensor_tensor)
    gain = exp(-comb)                          (ACT)
which equals exp(c*q(d)) of the reference knee exactly.  out_c = gain * x_c
is computed in place into the input tiles, which are then DMA'd out
contiguously.  Ln/Exp/Square all live in the natural_log_exp_and_others
ACT table set; get_activation_tables is narrowed during compile so the
set chooser picks it (avoids per-chunk table reloads).
"""

import numpy as np

N, C, L = 32, 2, 131072
NCORES = 8
NE = N // NCORES          # examples per core
P = 128                   # partitions = chunks per example
F = L // P                # 1024 samples per partition
BANK = 512                # psum bank width for the carry fix
EPS = 1e-5

_CACHE = {}


def _build(nb):
    import concourse.bass as bass
    import concourse.tile as tile
    from concourse import bacc, mybir

    f32 = mybir.dt.float32
    AF = mybir.ActivationFunctionType
    OP = mybir.AluOpType

    nc = bacc.Bacc("TRN2", target_bir_lowering=False, debug=False, num_devices=NCORES)

    x_h = nc.declare_dram_parameter("x", [NE, C, L], f32, isOutput=False)
    scal_h = nc.declare_dram_parameter("scal", [P, 16 * NE], f32, isOutput=False)
    mmt_h = nc.declare_dram_parameter("mmt", [NE * P, P], f32, isOutput=False)
    dec_h = nc.declare_dram_parameter("decay", [1, NE * nb * BANK], f32, isOutput=False)
    out_h = nc.declare_dram_parameter("out", [NE, C, L], f32, isOutput=True)

    from contextlib import ExitStack

    with tile.TileContext(nc) as tc, ExitStack() as ctx:
        const = ctx.enter_context(tc.tile_pool(name="const", bufs=1))
        work = ctx.enter_context(tc.tile_pool(name="work", bufs=3))
        ypool = ctx.enter_context(tc.tile_pool(name="ypool", bufs=3))
        xpool = ctx.enter_context(tc.tile_pool(name="xpool", bufs=4))
        psum = ctx.enter_context(tc.tile_pool(name="psum", bufs=2, space="PSUM"))

        scal_t = const.tile([P, 16 * NE], f32)
        nc.sync.dma_start(scal_t[:], scal_h[:])
        mmt_t = [const.tile([P, P], f32, name=f"mmt{e}") for e in range(NE)]
        for e in range(NE):
            nc.sync.dma_start(mmt_t[e][:], mmt_h[:][e * P : (e + 1) * P, :])
        dec_t = const.tile([1, NE * nb * BANK], f32, padded_shape=[P, NE * nb * BANK])
        nc.sync.dma_start(dec_t[:], dec_h[:])

        def sc(e, j):
            return scal_t[:, 16 * e + j : 16 * e + j + 1]

        for e in range(NE):
            x0 = xpool.tile([P, F], f32, tag="x0")
            x1 = xpool.tile([P, F], f32, tag="x1")
            nc.gpsimd.dma_start(x0[:], x_h[:][e, 0].rearrange("(p i) -> p i", p=P))
            nc.gpsimd.dma_start(x1[:], x_h[:][e, 1].rearrange("(p i) -> p i", p=P))

            # energy e[t] = ((1-a)/2) * (x0^2 + x1^2), scale folded into the squares
            sq0 = work.tile([P, F], f32, tag="sq0")
            sq1 = work.tile([P, F], f32, tag="sq1")
            nc.scalar.activation(sq0[:], x0[:], AF.Square, bias=0.0, scale=sc(e, 0))
            nc.scalar.activation(sq1[:], x1[:], AF.Square, bias=0.0, scale=sc(e, 0))
            e_t = work.tile([P, F], f32, tag="e")
            nc.vector.tensor_tensor(e_t[:], sq0[:], sq1[:], op=OP.add)

            # local scans, one independent recurrence per partition-chunk
            y_t = ypool.tile([P, F], f32, tag="y")
            nc.vector.tensor_tensor_scan(
                y_t[:], sc(e, 1).broadcast_to([P, F]), e_t[:], 0.0,
                op0=OP.mult, op1=OP.add,
            )

            # carry fix: carryT = S^T @ M^T, then y[:, :nb*512] += carry x decay
            p1 = psum.tile([1, P], f32, tag="p1")
            nc.tensor.matmul(p1[:], y_t[:, F - 1 : F], mmt_t[e][:],
                             start=True, stop=True)
            carryT = work.tile([1, P], f32, tag="carryT", padded_shape=[P, P])
            nc.scalar.copy(carryT[:], p1[:])
            for b in range(nb):
                pc = psum.tile([P, BANK], f32, tag="pc")
                off = (e * nb + b) * BANK
                nc.tensor.matmul(
                    pc[:], carryT[:], dec_t[0:1, off : off + BANK],
                    start=True, stop=True,
                )
                ysl = y_t[:, b * BANK : (b + 1) * BANK]
                nc.vector.tensor_add(ysl, ysl, pc[:])

            # knee gain
            d_t = work.tile([P, F], f32, tag="d")
            nc.scalar.activation(d_t[:], y_t[:], AF.Ln, bias=sc(e, 3), scale=sc(e, 2))
            u_t = work.tile([P, F], f32, tag="u")
            nc.vector.tensor_scalar(u_t[:], d_t[:], sc(e, 4), sc(e, 5),
                                    op0=OP.max, op1=OP.min)
            sqv_t = work.tile([P, F], f32, tag="sqv")
            nc.scalar.activation(sqv_t[:], u_t[:], AF.Square,
                                 bias=sc(e, 8), scale=sc(e, 7))
            dm_t = work.tile([P, F], f32, tag="dm")
            nc.vector.tensor_scalar(dm_t[:], d_t[:], sc(e, 5), sc(e, 6),
                                    op0=OP.subtract, op1=OP.mult)
            comb_t = work.tile([P, F], f32, tag="comb")
            nc.vector.scalar_tensor_tensor(comb_t[:], dm_t[:], 0.0, sqv_t[:],
                                           op0=OP.max, op1=OP.add)
            g_t = work.tile([P, F], f32, tag="g")
            nc.scalar.activation(g_t[:], comb_t[:], AF.Exp, bias=0.0, scale=-1.0)

            # gain application in place, then contiguous DMA out
            nc.vector.tensor_mul(x0[:], g_t[:], x0[:])
            nc.gpsimd.tensor_tensor(x1[:], g_t[:], x1[:], op=OP.mult)
            nc.sync.dma_start(out_h[:][e, 0].rearrange("(p i) -> p i", p=P), x0[:])
            nc.sync.dma_start(out_h[:][e, 1].rearrange("(p i) -> p i", p=P), x1[:])

    # narrow the ACT table sets so Ln/Exp/Square resolve to the one set that
    # holds all three -> a single table load instead of per-chunk reloads
    import concourse.bacc as bacc_mod

    orig = bacc_mod.get_activation_tables
    strip = {AF.Ln, AF.Exp, AF.Square}

    def patched(arch):
        full = orig(arch)
        return {
            name: (set(fns) if name == "natural_log_exp_and_others"
                   else set(fns) - strip)
            for name, fns in full.items()
        }

    bacc_mod.get_activation_tables = patched
    try:
        nc.compile()
    finally:
        bacc_mod.get_activation_tables = orig
    return nc


def _host_consts(lt, lr, lk, za, nb):
    """Per-core constant tensors from the [NE] parameter vectors (f64 math)."""
    alpha = 1.0 / (1.0 + np.exp(-za))
    thr = lt - 6.0
    r = 1.0 + np.exp(lr)
    c = 1.0 / r - 1.0
    W = np.exp(lk) / 2.0

    cols = np.zeros((NE, 16))
    cols[:, 0] = np.sqrt((1.0 - alpha) / 2.0)     # s1: energy scale
    cols[:, 1] = alpha
    cols[:, 2] = np.exp(-thr)                     # lnscale
    cols[:, 3] = EPS * np.exp(-thr)               # lnbias
    cols[:, 4] = -W
    cols[:, 5] = W
    cols[:, 6] = -c
    cols[:, 7] = np.sqrt(-c / (4.0 * W))          # s
    cols[:, 8] = np.sqrt(-c / (4.0 * W)) * W      # s*W
    scal = np.tile(cols.reshape(1, NE * 16), (P, 1)).astype(np.float32)

    # carry matrix, transposed for the matmul: mmt[e][q, p] = A^(p-1-q), q < p
    A = alpha**F
    mmt = np.zeros((NE, P, P))
    qs = np.arange(P)
    for e in range(NE):
        for p in range(1, P):
            mmt[e, :p, p] = A[e] ** (p - 1 - qs[:p])
    mmt = mmt.reshape(NE * P, P).astype(np.float32)

    dec = (alpha[:, None] ** np.arange(1, nb * BANK + 1)[None, :]).astype(np.float32)
    return {"scal": scal, "mmt": mmt, "decay": dec.reshape(1, NE * nb * BANK)}


def _pick_nb(za):
    alpha_max = float(1.0 / (1.0 + np.exp(-np.max(za))))
    alpha_max = min(max(alpha_max, 1e-6), 1.0 - 1e-9)
    need = np.log(1e-10) / np.log(alpha_max)
    return int(min(max(np.ceil(need / BANK), 1), F // BANK))


def _prep(inputs):
    x = np.ascontiguousarray(np.asarray(inputs["input_signals"], np.float32))
    lt = np.asarray(inputs["log_threshold"], np.float64).reshape(N)
    lr = np.asarray(inputs["log_ratio"], np.float64).reshape(N)
    lk = np.asarray(inputs["log_knee"], np.float64).reshape(N)
    za = np.asarray(inputs["z_alpha_pre"], np.float64).reshape(N)
    nb = _pick_nb(za)
    in_maps = []
    for i in range(NCORES):
        s = slice(i * NE, (i + 1) * NE)
        m = {"x": x[s]}
        m.update(_host_consts(lt[s], lr[s], lk[s], za[s], nb))
        in_maps.append(m)
    return nb, in_maps


def _get_nc(nb):
    if nb not in _CACHE:
        _CACHE[nb] = _build(nb)
    return _CACHE[nb]


def _run(inputs, trace=False):
    from concourse.bass_utils import run_bass_kernel_spmd

    nb, in_maps = _prep(inputs)
    nc = _get_nc(nb)
    res = run_bass_kernel_spmd(nc, in_maps, core_ids=list(range(NCORES)), trace=trace)
    out = np.concatenate([res.results[i]["out"] for i in range(NCORES)], axis=0)
    return out, res


def kernel(**inputs):
    out, _ = _run(inputs, trace=False)
    return out
